# revision 22
# baseline (speedup 1.0000x reference)
"""Trainium2 Bass kernel for pre-LN multi-head attention with null-KV.

Computation (fp32 reference):
    xn = LayerNorm(x) * gamma
    q = (xn @ w_q) * scale ; k, v = split(xn @ w_kv)
    k, v prepended with per-head null_kv
    out = softmax(q k^T) v   (16 heads, dim 64)
    y = out @ w_out

Sharding: 8 cores = 4 batches x 2 head-groups (8 heads each). Each core
computes LN for its batch, projections for its head group (gamma and the
q-scale folded into the weights on the host), attention, and a partial
out-projection; the host sums the two partial outputs per batch.

On-device layout is fully "transposed": xn is normalized to fp16 on the
gpsimd engine, transposed on the PE (fp16 transposes run 4x faster than
fp32), projections produce q^T / k^T directly, sim is computed as k q^T
(keys on partitions) so exp needs no cross-partition reductions, and
attn@v is computed with v (keys on partitions) as the stationary operand
with a fused ones-column yielding the softmax denominator in the same
matmul. The null key/value contribute one K=1 accumulation per head.
All matmul operands are fp16 (fp8 fails the accuracy budget: attention
output is itself an average, so quantization noise does not wash out
relative to the signal).
"""

import sys

sys.path.insert(0, "/opt/trn_rl_repo")

import numpy as np

HEADS = 16
DIM_HEAD = 64
DIM = 1024
INNER = HEADS * DIM_HEAD
SCALE = DIM_HEAD ** -0.5

N_TOK = 2048      # sequence length per batch
HC = 512          # head-cols per core (8 heads x 64)
NHEAD = 8         # heads per core
NPAIR = 4         # head pairs per core
NKT = 16          # key tiles of 128
NQC = 4           # query chunks of 512
NQUARTER = 4      # token quarters of 512 for the pre-phase
KC = 8            # contraction chunks of 128 over DIM

_CACHE: dict = {}


def _build_nc():
    from contextlib import ExitStack

    import concourse.bacc as bacc
    import concourse.bass as bass
    import concourse.tile as tile
    from concourse import mybir

    f32 = mybir.dt.float32
    f16 = mybir.dt.float16
    AF = mybir.ActivationFunctionType
    ALU = mybir.AluOpType
    PSUM = bass.MemorySpace.PSUM

    nc = bacc.Bacc(None)

    x_d = nc.declare_dram_parameter("x", [N_TOK, DIM], f32, isOutput=False)
    wq_d = nc.declare_dram_parameter("wq", [DIM, HC], f16, isOutput=False)
    wk_d = nc.declare_dram_parameter("wk", [DIM, HC], f16, isOutput=False)
    wv_d = nc.declare_dram_parameter("wv", [DIM, HC], f16, isOutput=False)
    wo_d = nc.declare_dram_parameter("wo", [HC, DIM], f16, isOutput=False)
    vones_d = nc.declare_dram_parameter("vones", [128, NHEAD], f16, isOutput=False)
    nullk_d = nc.declare_dram_parameter("nullk", [128, NHEAD], f16, isOutput=False)
    nullv_d = nc.declare_dram_parameter("nullv", [1, NHEAD, 65], f16, isOutput=False)
    ident_d = nc.declare_dram_parameter("ident", [128, 128], f16, isOutput=False)
    y_d = nc.declare_dram_parameter("y", [N_TOK, DIM], f32, isOutput=True)

    with tile.TileContext(nc) as tc, ExitStack() as ctx:
        pers = ctx.enter_context(tc.tile_pool(name="pers", bufs=1))

        ident_sb = pers.tile([128, 128], f16, tag="ident", name="ident")
        nc.sync.dma_start(out=ident_sb, in_=ident_d[:, :])
        eps_sb = pers.tile([128, 1], f32, tag="eps", name="eps")
        nc.vector.memset(eps_sb, 1e-5)

        nullk_sb = pers.tile([128, NHEAD], f16, tag="nullk", name="nullk")
        nc.sync.dma_start(out=nullk_sb, in_=nullk_d[:, :])
        nullv_sb = pers.tile([1, NHEAD, 65], f16, tag="nullv", name="nullv")
        nc.sync.dma_start(out=nullv_sb, in_=nullv_d[:, :, :])
        pnall = [pers.tile([1, 2, NQC, 512], f16, tag=f"pna{i}", name=f"pna{i}")
                 for i in range(NPAIR)]
        qT = [pers.tile([128, N_TOK], f16, tag=f"qT{i}", name=f"qT{i}") for i in range(NPAIR)]
        kT = [pers.tile([128, N_TOK], f16, tag=f"kT{i}", name=f"kT{i}") for i in range(NPAIR)]
        # flat [128, 584]: head h's v in cols h*65..h*65+64, the fused ones
        # column at h*65+64; 584 wide so a 128-col lhsT slice (FWL-eligible)
        # stays in bounds for every head
        v_sb = [pers.tile([128, 584], f16, tag=f"v{i}", name=f"v{i}") for i in range(NKT)]
        for t in range(NKT):
            nc.sync.dma_start(
                out=v_sb[t][:, 0:520].rearrange("p (h e) -> p h e", e=65)[:, :, 64:65],
                in_=vones_d[:, :])

        # ---------------- pre-phase: LN + transpose + projections ----------
        with (
            tc.tile_pool(name="wpool", bufs=1) as wpool,
            tc.tile_pool(name="xpool", bufs=8) as xpool,
            tc.tile_pool(name="xnpool", bufs=5) as xnpool,
            tc.tile_pool(name="xnt", bufs=3) as xnt_pool,
            tc.tile_pool(name="stat", bufs=4) as stat,
            tc.tile_pool(name="tps", bufs=4, space=PSUM) as tps,
            tc.tile_pool(name="pps", bufs=3, space=PSUM) as pps,
        ):
            # queue the first quarter's x tiles before the weight DMAs so
            # the LN/transpose pipeline starts immediately
            def dma_x(x_t, tt):
                # split the 512KB tile across 4 DMA queues: a single-queue
                # transfer is ~23us, which would serialize the kernel start
                rows = slice(tt * 128, (tt + 1) * 128)
                for i in range(4):
                    cs = slice(i * 256, (i + 1) * 256)
                    nc.sync.dma_start(out=x_t[:, cs], in_=x_d[rows, cs])

            xq0 = []
            for t4 in range(4):
                x_t = xpool.tile([128, DIM], f32, tag="x", name="x0")
                dma_x(x_t, t4)
                xq0.append(x_t)
            wq_sb = [wpool.tile([128, HC], f16, tag=f"wq{k}", name=f"wq{k}") for k in range(KC)]
            wk_sb = [wpool.tile([128, HC], f16, tag=f"wk{k}", name=f"wk{k}") for k in range(KC)]
            wv_sb = [wpool.tile([128, HC], f16, tag=f"wv{k}", name=f"wv{k}") for k in range(KC)]
            for k in range(KC):
                sl = slice(k * 128, (k + 1) * 128)
                nc.sync.dma_start(out=wq_sb[k], in_=wq_d[sl, :])
                nc.sync.dma_start(out=wk_sb[k], in_=wk_d[sl, :])
            for k in range(KC):
                sl = slice(k * 128, (k + 1) * 128)
                nc.sync.dma_start(out=wv_sb[k], in_=wv_d[sl, :])

            def emit_nulls(Qn):
                # null-key logits for quarter Qn, emitted one quarter late so
                # the qT drain copies they read are long finished
                for ct in range(NPAIR):
                    for h2 in range(2):
                        h = ct * 2 + h2
                        ro = h2 * 64
                        pnp = pps.tile([128, 512], f32, tag="proj",
                                       name="pnp")
                        nc.tensor.matmul(
                            pnp[0:1, :],
                            lhsT=nullk_sb[ro:ro + 64, h:h + 1],
                            rhs=qT[ct][ro:ro + 64,
                                       Qn * 512:(Qn + 1) * 512],
                            start=True, stop=True)
                        nc.scalar.activation(
                            out=pnall[ct][0:1, h2, Qn, :],
                            in_=pnp[0:1, :], func=AF.Exp)

            for Q in range(NQUARTER):
                qcols = slice(Q * 512, (Q + 1) * 512)
                if Q > 0:
                    emit_nulls(Q - 1)
                xns = []
                for t4 in range(4):
                    tt = Q * 4 + t4
                    if Q == 0:
                        x_t = xq0[t4]
                    else:
                        x_t = xpool.tile([128, DIM], f32, tag="x", name="x")
                        dma_x(x_t, tt)
                    st6 = stat.tile([128, 2, 6], f32, tag="st6", name="st6")
                    nc.vector.bn_stats(out=st6[:, 0, :], in_=x_t[:, 0:512])
                    nc.vector.bn_stats(out=st6[:, 1, :], in_=x_t[:, 512:1024])
                    mv = stat.tile([128, 2], f32, tag="mv", name="mv")
                    nc.vector.bn_aggr(out=mv, in_=st6)
                    rstd = stat.tile([128, 1], f32, tag="rstd", name="rstd")
                    nc.scalar.activation(out=rstd, in_=mv[:, 1:2], func=AF.Sqrt,
                                         bias=eps_sb, scale=1.0)
                    nc.vector.reciprocal(out=rstd, in_=rstd)
                    xn_t = xnpool.tile([128, DIM], f16, tag="xn", name="xn")
                    nc.vector.tensor_scalar(out=xn_t, in0=x_t,
                                            scalar1=mv[:, 0:1], scalar2=rstd,
                                            op0=ALU.subtract, op1=ALU.mult)
                    xns.append(xn_t)

                # fp16 transposes, batched 4 token-tiles per psum tile so the
                # drain copy is [128, 512]
                xnT = [xnt_pool.tile([128, 512], f16, tag=f"xnT{k}",
                                     name=f"xnT{k}") for k in range(KC)]
                for fc in range(KC):
                    tp = tps.tile([128, 512], f16, tag="tp", name="tp")
                    for t4 in range(4):
                        nc.tensor.transpose(
                            tp[:, t4 * 128:(t4 + 1) * 128],
                            xns[t4][:, fc * 128:(fc + 1) * 128], ident_sb)
                    if fc % 2 == 0:
                        nc.scalar.copy(out=xnT[fc], in_=tp)
                    else:
                        nc.vector.tensor_copy(out=xnT[fc], in_=tp)

                # q^T and k^T projections for this token quarter
                for w_sb, dst, eng in ((wq_sb, qT, 0), (wk_sb, kT, 1)):
                    for ct in range(NPAIR):
                        ps = pps.tile([128, 512], f32, tag="proj", name="proj")
                        for k in range(KC):
                            nc.tensor.matmul(
                                ps,
                                lhsT=w_sb[k][:, ct * 128:(ct + 1) * 128],
                                rhs=xnT[k],
                                start=(k == 0), stop=(k == KC - 1))
                        if (ct + eng) % 2 == 0:
                            nc.vector.tensor_copy(out=dst[ct][:, qcols], in_=ps)
                        else:
                            nc.scalar.copy(out=dst[ct][:, qcols], in_=ps)
                # v projection (natural layout)
                for t4 in range(4):
                    tt = Q * 4 + t4
                    ps = pps.tile([128, 512], f32, tag="proj", name="projv")
                    for k in range(KC):
                        nc.tensor.matmul(
                            ps,
                            lhsT=xnT[k][:, t4 * 128:(t4 + 1) * 128],
                            rhs=wv_sb[k],
                            start=(k == 0), stop=(k == KC - 1))
                    nc.vector.tensor_copy(
                        out=v_sb[tt][:, 0:520].rearrange(
                            "p (h e) -> p h e", e=65)[:, :, 0:64],
                        in_=ps.rearrange("p (h d) -> p h d", h=NHEAD))

            emit_nulls(NQUARTER - 1)

        # ---------------- attention + out-projection ------------------------
        # Unit = 2 consecutive key tiles x both heads of a pair. Sims land in
        # 2-bank PSUM tiles (two matmuls share one tile-level semaphore) and
        # exp covers [128, 1024] per head, so PE-side waits are halved; the
        # attn@v stage trails by LAG units so the PE never blocks on exp.
        LAG = 1
        NU = NKT // 2
        with (
            tc.tile_pool(name="apool", bufs=1) as apool,
            tc.tile_pool(name="ppool", bufs=7) as ppool,
            tc.tile_pool(name="rcpool", bufs=2) as rcpool,
            tc.tile_pool(name="rbpool", bufs=3) as rbpool,
            tc.tile_pool(name="aspool", bufs=3) as aspool,
            tc.tile_pool(name="ypool", bufs=3) as ypool,
            tc.tile_pool(name="simy", bufs=3, space=PSUM) as simy,
            tc.tile_pool(name="accp", bufs=2, space=PSUM) as accps,
        ):
            outT = [apool.tile([128, N_TOK], f16, tag=f"outT{i}", name=f"outT{i}")
                    for i in range(NPAIR)]
            wo_sb = [apool.tile([128, DIM], f16, tag=f"wo{i}", name=f"wo{i}")
                     for i in range(NPAIR)]
            for i in range(NPAIR):
                nc.sync.dma_start(out=wo_sb[i], in_=wo_d[i * 128:(i + 1) * 128, :])

            def emit_yproj(c, t4):
                tt = c * 4 + t4
                for nh in range(2):
                    yp = simy.tile([128, 1024], f32, tag="simy", name="yp")
                    for ic in range(NPAIR):
                        nc.tensor.matmul(
                            yp[:, 0:512],
                            lhsT=outT[ic][:, tt * 128:(tt + 1) * 128],
                            rhs=wo_sb[ic][:, nh * 512:(nh + 1) * 512],
                            start=(ic == 0), stop=(ic == NPAIR - 1))
                    y_sb = ypool.tile([128, 512], f32, tag="ysb", name="ysb")
                    nc.vector.tensor_copy(out=y_sb, in_=yp[:, 0:512])
                    for i in range(2):
                        cs = slice(i * 256, (i + 1) * 256)
                        nc.sync.dma_start(
                            out=y_d[tt * 128:(tt + 1) * 128,
                                    nh * 512 + i * 256:nh * 512 + (i + 1) * 256],
                            in_=y_sb[:, cs])

            # accs are zeroed on the vector engine and every matmul
            # accumulates with start=False: the accumulation commutes, so
            # emission order around it is free — the null-kv init is emitted
            # late (after unit 0's sims) with the yproj matmuls as PE filler
            # during the first exp's latency, and each iteration's memsets
            # are hoisted into the previous iteration's tail so the DVE has
            # them done before the PE needs the acc.
            def fresh_accs():
                accs = []
                for h2 in range(2):
                    acc = accps.tile([128, 512], f32, tag="acc", name="acc")
                    nc.vector.memset(acc, 0.0)
                    accs.append(acc)
                return accs

            next_accs = fresh_accs()
            for c in range(NQC):
                ccols = slice(c * 512, (c + 1) * 512)
                for pr in range(NPAIR):
                    accs = next_accs
                    pend = []
                    for u in range(NU + LAG):
                        if u < NU:
                            sims = [simy.tile([128, 1024], f32, tag="simy",
                                              name="sim") for _ in range(2)]
                            for j in range(2):
                                kt = u * 2 + j
                                for h2 in range(2):
                                    ro = h2 * 64
                                    nc.tensor.matmul(
                                        sims[h2][:, j * 512:(j + 1) * 512],
                                        lhsT=kT[pr][ro:ro + 64,
                                                    kt * 128:(kt + 1) * 128],
                                        rhs=qT[pr][ro:ro + 64, ccols],
                                        start=True, stop=True)
                            ps2 = []
                            for h2 in range(2):
                                p_sb = ppool.tile([128, 1024], f16, tag="P",
                                                  name="P")
                                nc.scalar.activation(out=p_sb, in_=sims[h2],
                                                     func=AF.Exp)
                                ps2.append(p_sb)
                            pend.append(ps2)
                        if u == 0:
                            if c > 0:
                                emit_yproj(c - 1, pr)
                            for h2 in range(2):
                                h = pr * 2 + h2
                                nc.tensor.matmul(
                                    accs[h2][0:65, :],
                                    lhsT=nullv_sb[0:1, h, :],
                                    rhs=pnall[pr][0:1, h2, c, :],
                                    start=False, stop=False)
                        if u >= LAG:
                            ud = u - LAG
                            ps2 = pend[ud]
                            for h2 in range(2):
                                h = pr * 2 + h2
                                for j in range(2):
                                    kt = ud * 2 + j
                                    nc.tensor.matmul(
                                        accs[h2],
                                        lhsT=v_sb[kt][:, h * 65:h * 65 + 128],
                                        rhs=ps2[h2][:, j * 512:(j + 1) * 512],
                                        start=False, stop=(kt == NKT - 1))
                    # softmax normalization, off the critical path
                    if not (c == NQC - 1 and pr == NPAIR - 1):
                        next_accs = fresh_accs()
                    stg = []
                    for h2 in range(2):
                        a_s = aspool.tile([65, 512], f32, tag="accS",
                                          name="accS")
                        nc.vector.tensor_copy(out=a_s, in_=accs[h2][0:65, :])
                        z0 = rcpool.tile([1, 512], f32, tag="z0", name="z0")
                        nc.vector.tensor_copy(out=z0, in_=a_s[64:65, :])
                        rc = rcpool.tile([1, 512], f32, tag="rc", name="rc")
                        nc.vector.reciprocal_approx_fast(out=rc, in_=z0)
                        stg.append((a_s, rc))
                    rbs = []
                    for h2 in range(2):
                        rb = rbpool.tile([64, 512], f32, tag="rb", name="rb")
                        nc.gpsimd.partition_broadcast(rb, stg[h2][1],
                                                      channels=64)
                        rbs.append(rb)
                    for h2 in range(2):
                        ro = h2 * 64
                        nc.vector.tensor_mul(
                            out=outT[pr][ro:ro + 64, ccols],
                            in0=stg[h2][0][0:64, :], in1=rbs[h2])
            for t4 in range(4):
                emit_yproj(NQC - 1, t4)

    nc.compile()
    return nc


def _get_nc():
    if "nc" not in _CACHE:
        _CACHE["nc"] = _build_nc()
    return _CACHE["nc"]


def _prep_in_maps(x, gamma, w_q, w_kv, w_out, null_kv):
    x = np.asarray(x, dtype=np.float32)
    gamma = np.asarray(gamma, dtype=np.float32)
    w_q = np.asarray(w_q, dtype=np.float32)
    w_kv = np.asarray(w_kv, dtype=np.float32)
    w_out = np.asarray(w_out, dtype=np.float32)
    null_kv = np.asarray(null_kv, dtype=np.float32)

    g = gamma[:, None]
    wq_full = g * w_q * SCALE
    wk_full = g * w_kv[:, :INNER]
    wv_full = g * w_kv[:, INNER:]
    ident = np.eye(128, dtype=np.float16)

    in_maps = []
    for core in range(8):
        b, gr = core // 2, core % 2
        hs = slice(gr * HC, (gr + 1) * HC)
        nullk = np.zeros((128, NHEAD), dtype=np.float16)
        nullv = np.zeros((1, NHEAD, 65), dtype=np.float16)
        for j in range(NHEAD):
            h = gr * NHEAD + j
            ro = (j % 2) * 64
            nullk[ro:ro + 64, j] = null_kv[0, h, 0, :]
            nullv[0, j, :64] = null_kv[1, h, 0, :]
            nullv[0, j, 64] = 1.0
        in_maps.append({
            "x": np.ascontiguousarray(x[b]),
            "vones": np.ones((128, NHEAD), dtype=np.float16),
            "wq": np.ascontiguousarray(wq_full[:, hs]).astype(np.float16),
            "wk": np.ascontiguousarray(wk_full[:, hs]).astype(np.float16),
            "wv": np.ascontiguousarray(wv_full[:, hs]).astype(np.float16),
            "wo": np.ascontiguousarray(w_out[hs, :]).astype(np.float16),
            "nullk": nullk,
            "nullv": nullv,
            "ident": ident,
        })
    return in_maps


def kernel(x, gamma, w_q, w_kv, w_out, null_kv, _want_results=False):
    from concourse.bass_utils import run_bass_kernel_spmd

    nc = _get_nc()
    in_maps = _prep_in_maps(x, gamma, w_q, w_kv, w_out, null_kv)
    res = run_bass_kernel_spmd(nc, in_maps, list(range(8)))
    outs = [res.results[c]["y"] for c in range(8)]
    y = np.empty((4, N_TOK, DIM), dtype=np.float32)
    for b in range(4):
        np.add(outs[2 * b], outs[2 * b + 1], out=y[b])
    if _want_results:
        return y, res
    return y


# revision 24
# speedup vs baseline: 1.0488x; 1.0488x over previous
"""Trainium2 Bass kernel for pre-LN multi-head attention with null-KV.

Computation (fp32 reference):
    xn = LayerNorm(x) * gamma
    q = (xn @ w_q) * scale ; k, v = split(xn @ w_kv)
    k, v prepended with per-head null_kv
    out = softmax(q k^T) v   (16 heads, dim 64)
    y = out @ w_out

Sharding: 8 cores = 4 batches x 2 head-groups (8 heads each). Each core
computes LN for its batch, projections for its head group (gamma and the
q-scale folded into the weights on the host), attention, and a partial
out-projection; the host sums the two partial outputs per batch.

On-device layout is fully "transposed": xn is normalized to fp16 on the
gpsimd engine, transposed on the PE (fp16 transposes run 4x faster than
fp32), projections produce q^T / k^T directly, sim is computed as k q^T
(keys on partitions) so exp needs no cross-partition reductions, and
attn@v is computed with v (keys on partitions) as the stationary operand
with a fused ones-column yielding the softmax denominator in the same
matmul. The null key/value contribute one K=1 accumulation per head.
All matmul operands are fp16 (fp8 fails the accuracy budget: attention
output is itself an average, so quantization noise does not wash out
relative to the signal).
"""

import sys

sys.path.insert(0, "/opt/trn_rl_repo")

import numpy as np

HEADS = 16
DIM_HEAD = 64
DIM = 1024
INNER = HEADS * DIM_HEAD
SCALE = DIM_HEAD ** -0.5

N_TOK = 2048      # sequence length per batch
HC = 512          # head-cols per core (8 heads x 64)
NHEAD = 8         # heads per core
NPAIR = 4         # head pairs per core
NKT = 16          # key tiles of 128
NQC = 4           # query chunks of 512
NQUARTER = 4      # token quarters of 512 for the pre-phase
KC = 8            # contraction chunks of 128 over DIM

_CACHE: dict = {}


def _build_nc():
    from contextlib import ExitStack

    import concourse.bacc as bacc
    import concourse.bass as bass
    import concourse.tile as tile
    from concourse import mybir

    f32 = mybir.dt.float32
    f16 = mybir.dt.float16
    AF = mybir.ActivationFunctionType
    ALU = mybir.AluOpType
    PSUM = bass.MemorySpace.PSUM

    nc = bacc.Bacc(None)

    x_d = nc.declare_dram_parameter("x", [N_TOK, DIM], f32, isOutput=False)
    wq_d = nc.declare_dram_parameter("wq", [DIM, HC], f16, isOutput=False)
    wk_d = nc.declare_dram_parameter("wk", [DIM, HC], f16, isOutput=False)
    wv_d = nc.declare_dram_parameter("wv", [DIM, HC], f16, isOutput=False)
    wo_d = nc.declare_dram_parameter("wo", [HC, DIM], f16, isOutput=False)
    nullk_d = nc.declare_dram_parameter("nullk", [128, NHEAD], f16, isOutput=False)
    nullv_d = nc.declare_dram_parameter("nullv", [1, NHEAD, 65], f16, isOutput=False)
    ident_d = nc.declare_dram_parameter("ident", [128, 128], f16, isOutput=False)
    y_d = nc.declare_dram_parameter("y", [N_TOK, DIM], f32, isOutput=True)

    with tile.TileContext(nc) as tc, ExitStack() as ctx:
        pers = ctx.enter_context(tc.tile_pool(name="pers", bufs=1))

        ident_sb = pers.tile([128, 128], f16, tag="ident", name="ident")
        nc.sync.dma_start(out=ident_sb, in_=ident_d[:, :])
        eps_sb = pers.tile([128, 1], f32, tag="eps", name="eps")
        nc.vector.memset(eps_sb, 1e-5)

        nullk_sb = pers.tile([128, NHEAD], f16, tag="nullk", name="nullk")
        nc.sync.dma_start(out=nullk_sb, in_=nullk_d[:, :])
        nullv_sb = pers.tile([1, NHEAD, 65], f16, tag="nullv", name="nullv")
        nc.sync.dma_start(out=nullv_sb, in_=nullv_d[:, :, :])
        pnall = [pers.tile([1, 2, NQC, 512], f16, tag=f"pna{i}", name=f"pna{i}")
                 for i in range(NPAIR)]
        qT = [pers.tile([128, N_TOK], f16, tag=f"qT{i}", name=f"qT{i}") for i in range(NPAIR)]
        kT = [pers.tile([128, N_TOK], f16, tag=f"kT{i}", name=f"kT{i}") for i in range(NPAIR)]
        # flat [128, 584]: head h's v in cols h*65..h*65+64, the fused ones
        # column at h*65+64; 584 wide so a 128-col lhsT slice (FWL-eligible)
        # stays in bounds for every head
        v_sb = [pers.tile([128, 584], f16, tag=f"v{i}", name=f"v{i}") for i in range(NKT)]
        for t in range(NKT):
            nc.vector.memset(
                v_sb[t][:, 0:520].rearrange("p (h e) -> p h e", e=65)[:, :, 64:65],
                1.0)

        # ---------------- pre-phase: LN + transpose + projections ----------
        with (
            tc.tile_pool(name="wpool", bufs=1) as wpool,
            tc.tile_pool(name="xpool", bufs=8) as xpool,
            tc.tile_pool(name="xnpool", bufs=5) as xnpool,
            tc.tile_pool(name="xnt", bufs=3) as xnt_pool,
            tc.tile_pool(name="stat", bufs=4) as stat,
            tc.tile_pool(name="tps", bufs=4, space=PSUM) as tps,
            tc.tile_pool(name="pps", bufs=3, space=PSUM) as pps,
        ):
            # queue the first quarter's x tiles before the weight DMAs so
            # the LN/transpose pipeline starts immediately
            def dma_x(x_t, tt):
                nc.sync.dma_start(out=x_t,
                                  in_=x_d[tt * 128:(tt + 1) * 128, :])

            xq0 = []
            for t4 in range(4):
                x_t = xpool.tile([128, DIM], f32, tag="x", name="x0")
                dma_x(x_t, t4)
                xq0.append(x_t)
            wq_sb = [wpool.tile([128, HC], f16, tag=f"wq{k}", name=f"wq{k}") for k in range(KC)]
            wk_sb = [wpool.tile([128, HC], f16, tag=f"wk{k}", name=f"wk{k}") for k in range(KC)]
            wv_sb = [wpool.tile([128, HC], f16, tag=f"wv{k}", name=f"wv{k}") for k in range(KC)]
            for k in range(KC):
                sl = slice(k * 128, (k + 1) * 128)
                nc.scalar.dma_start(out=wq_sb[k], in_=wq_d[sl, :])
                nc.scalar.dma_start(out=wk_sb[k], in_=wk_d[sl, :])
            for k in range(KC):
                sl = slice(k * 128, (k + 1) * 128)
                nc.scalar.dma_start(out=wv_sb[k], in_=wv_d[sl, :])

            def emit_nulls(Qn):
                # null-key logits for quarter Qn, emitted one quarter late so
                # the qT drain copies they read are long finished
                for ct in range(NPAIR):
                    for h2 in range(2):
                        h = ct * 2 + h2
                        ro = h2 * 64
                        pnp = pps.tile([128, 512], f32, tag="proj",
                                       name="pnp")
                        nc.tensor.matmul(
                            pnp[0:1, :],
                            lhsT=nullk_sb[ro:ro + 64, h:h + 1],
                            rhs=qT[ct][ro:ro + 64,
                                       Qn * 512:(Qn + 1) * 512],
                            start=True, stop=True)
                        nc.scalar.activation(
                            out=pnall[ct][0:1, h2, Qn, :],
                            in_=pnp[0:1, :], func=AF.Exp)

            for Q in range(NQUARTER):
                qcols = slice(Q * 512, (Q + 1) * 512)
                if Q > 0:
                    emit_nulls(Q - 1)
                xns = []
                for t4 in range(4):
                    tt = Q * 4 + t4
                    if Q == 0:
                        x_t = xq0[t4]
                    else:
                        x_t = xpool.tile([128, DIM], f32, tag="x", name="x")
                        dma_x(x_t, tt)
                    st6 = stat.tile([128, 2, 6], f32, tag="st6", name="st6")
                    nc.vector.bn_stats(out=st6[:, 0, :], in_=x_t[:, 0:512])
                    nc.vector.bn_stats(out=st6[:, 1, :], in_=x_t[:, 512:1024])
                    mv = stat.tile([128, 2], f32, tag="mv", name="mv")
                    nc.vector.bn_aggr(out=mv, in_=st6)
                    rstd = stat.tile([128, 1], f32, tag="rstd", name="rstd")
                    nc.scalar.activation(out=rstd, in_=mv[:, 1:2], func=AF.Sqrt,
                                         bias=eps_sb, scale=1.0)
                    nc.vector.reciprocal(out=rstd, in_=rstd)
                    xn_t = xnpool.tile([128, DIM], f16, tag="xn", name="xn")
                    nc.vector.tensor_scalar(out=xn_t, in0=x_t,
                                            scalar1=mv[:, 0:1], scalar2=rstd,
                                            op0=ALU.subtract, op1=ALU.mult)
                    xns.append(xn_t)

                # fp16 transposes, batched 4 token-tiles per psum tile so the
                # drain copy is [128, 512]
                xnT = [xnt_pool.tile([128, 512], f16, tag=f"xnT{k}",
                                     name=f"xnT{k}") for k in range(KC)]
                for fc in range(KC):
                    tp = tps.tile([128, 512], f16, tag="tp", name="tp")
                    for t4 in range(4):
                        nc.tensor.transpose(
                            tp[:, t4 * 128:(t4 + 1) * 128],
                            xns[t4][:, fc * 128:(fc + 1) * 128], ident_sb)
                    if fc % 2 == 0:
                        nc.scalar.copy(out=xnT[fc], in_=tp)
                    else:
                        nc.vector.tensor_copy(out=xnT[fc], in_=tp)

                # q^T and k^T projections for this token quarter
                for w_sb, dst, eng in ((wq_sb, qT, 0), (wk_sb, kT, 1)):
                    for ct in range(NPAIR):
                        ps = pps.tile([128, 512], f32, tag="proj", name="proj")
                        for k in range(KC):
                            nc.tensor.matmul(
                                ps,
                                lhsT=w_sb[k][:, ct * 128:(ct + 1) * 128],
                                rhs=xnT[k],
                                start=(k == 0), stop=(k == KC - 1))
                        if (ct + eng) % 2 == 0:
                            nc.vector.tensor_copy(out=dst[ct][:, qcols], in_=ps)
                        else:
                            nc.scalar.copy(out=dst[ct][:, qcols], in_=ps)
                # v projection (natural layout)
                for t4 in range(4):
                    tt = Q * 4 + t4
                    ps = pps.tile([128, 512], f32, tag="proj", name="projv")
                    for k in range(KC):
                        nc.tensor.matmul(
                            ps,
                            lhsT=xnT[k][:, t4 * 128:(t4 + 1) * 128],
                            rhs=wv_sb[k],
                            start=(k == 0), stop=(k == KC - 1))
                    nc.vector.tensor_copy(
                        out=v_sb[tt][:, 0:520].rearrange(
                            "p (h e) -> p h e", e=65)[:, :, 0:64],
                        in_=ps.rearrange("p (h d) -> p h d", h=NHEAD))

            emit_nulls(NQUARTER - 1)

        # ---------------- attention + out-projection ------------------------
        # Unit = 2 consecutive key tiles x both heads of a pair. Sims land in
        # 2-bank PSUM tiles (two matmuls share one tile-level semaphore) and
        # exp covers [128, 1024] per head, so PE-side waits are halved; the
        # attn@v stage trails by LAG units so the PE never blocks on exp.
        LAG = 1
        NU = NKT // 2
        with (
            tc.tile_pool(name="apool", bufs=1) as apool,
            tc.tile_pool(name="ppool", bufs=7) as ppool,
            tc.tile_pool(name="rcpool", bufs=2) as rcpool,
            tc.tile_pool(name="rbpool", bufs=3) as rbpool,
            tc.tile_pool(name="aspool", bufs=3) as aspool,
            tc.tile_pool(name="ypool", bufs=3) as ypool,
            tc.tile_pool(name="simy", bufs=3, space=PSUM) as simy,
            tc.tile_pool(name="accp", bufs=2, space=PSUM) as accps,
        ):
            outT = [apool.tile([128, N_TOK], f16, tag=f"outT{i}", name=f"outT{i}")
                    for i in range(NPAIR)]
            wo_sb = [apool.tile([128, DIM], f16, tag=f"wo{i}", name=f"wo{i}")
                     for i in range(NPAIR)]
            for i in range(NPAIR):
                nc.sync.dma_start(out=wo_sb[i], in_=wo_d[i * 128:(i + 1) * 128, :])

            def emit_yproj(c, t4):
                tt = c * 4 + t4
                for nh in range(2):
                    yp = simy.tile([128, 1024], f32, tag="simy", name="yp")
                    for ic in range(NPAIR):
                        nc.tensor.matmul(
                            yp[:, 0:512],
                            lhsT=outT[ic][:, tt * 128:(tt + 1) * 128],
                            rhs=wo_sb[ic][:, nh * 512:(nh + 1) * 512],
                            start=(ic == 0), stop=(ic == NPAIR - 1))
                    y_sb = ypool.tile([128, 512], f32, tag="ysb", name="ysb")
                    nc.vector.tensor_copy(out=y_sb, in_=yp[:, 0:512])
                    nc.sync.dma_start(
                        out=y_d[tt * 128:(tt + 1) * 128,
                                nh * 512:(nh + 1) * 512],
                        in_=y_sb)

            # accs are zeroed on the vector engine and every matmul
            # accumulates with start=False: the accumulation commutes, so
            # emission order around it is free — the null-kv init is emitted
            # late (after unit 0's sims) with the yproj matmuls as PE filler
            # during the first exp's latency, and each iteration's memsets
            # are hoisted into the previous iteration's tail so the DVE has
            # them done before the PE needs the acc.
            def fresh_accs():
                accs = []
                for h2 in range(2):
                    acc = accps.tile([128, 512], f32, tag="acc", name="acc")
                    nc.vector.memset(acc, 0.0)
                    accs.append(acc)
                return accs

            next_accs = fresh_accs()
            for c in range(NQC):
                ccols = slice(c * 512, (c + 1) * 512)
                for pr in range(NPAIR):
                    accs = next_accs
                    pend = []
                    for u in range(NU + LAG):
                        if u < NU:
                            sims = [simy.tile([128, 1024], f32, tag="simy",
                                              name="sim") for _ in range(2)]
                            for j in range(2):
                                kt = u * 2 + j
                                for h2 in range(2):
                                    ro = h2 * 64
                                    nc.tensor.matmul(
                                        sims[h2][:, j * 512:(j + 1) * 512],
                                        lhsT=kT[pr][ro:ro + 64,
                                                    kt * 128:(kt + 1) * 128],
                                        rhs=qT[pr][ro:ro + 64, ccols],
                                        start=True, stop=True)
                            ps2 = []
                            for h2 in range(2):
                                p_sb = ppool.tile([128, 1024], f16, tag="P",
                                                  name="P")
                                nc.scalar.activation(out=p_sb, in_=sims[h2],
                                                     func=AF.Exp)
                                ps2.append(p_sb)
                            pend.append(ps2)
                        if u == 0:
                            if c > 0:
                                emit_yproj(c - 1, pr)
                            for h2 in range(2):
                                h = pr * 2 + h2
                                nc.tensor.matmul(
                                    accs[h2][0:65, :],
                                    lhsT=nullv_sb[0:1, h, :],
                                    rhs=pnall[pr][0:1, h2, c, :],
                                    start=False, stop=False)
                        if u >= LAG:
                            ud = u - LAG
                            ps2 = pend[ud]
                            for h2 in range(2):
                                h = pr * 2 + h2
                                for j in range(2):
                                    kt = ud * 2 + j
                                    nc.tensor.matmul(
                                        accs[h2],
                                        lhsT=v_sb[kt][:, h * 65:h * 65 + 128],
                                        rhs=ps2[h2][:, j * 512:(j + 1) * 512],
                                        start=False, stop=(kt == NKT - 1))
                    # softmax normalization, off the critical path
                    if not (c == NQC - 1 and pr == NPAIR - 1):
                        next_accs = fresh_accs()
                    stg = []
                    for h2 in range(2):
                        a_s = aspool.tile([65, 512], f32, tag="accS",
                                          name="accS")
                        nc.vector.tensor_copy(out=a_s, in_=accs[h2][0:65, :])
                        z0 = rcpool.tile([1, 512], f32, tag="z0", name="z0")
                        nc.vector.tensor_copy(out=z0, in_=a_s[64:65, :])
                        rc = rcpool.tile([1, 512], f32, tag="rc", name="rc")
                        nc.vector.reciprocal_approx_fast(out=rc, in_=z0)
                        stg.append((a_s, rc))
                    rbs = []
                    for h2 in range(2):
                        rb = rbpool.tile([64, 512], f32, tag="rb", name="rb")
                        nc.gpsimd.partition_broadcast(rb, stg[h2][1],
                                                      channels=64)
                        rbs.append(rb)
                    for h2 in range(2):
                        ro = h2 * 64
                        nc.vector.tensor_mul(
                            out=outT[pr][ro:ro + 64, ccols],
                            in0=stg[h2][0][0:64, :], in1=rbs[h2])
            for t4 in range(4):
                emit_yproj(NQC - 1, t4)

    nc.compile()
    return nc


def _get_nc():
    if "nc" not in _CACHE:
        _CACHE["nc"] = _build_nc()
    return _CACHE["nc"]


def _prep_in_maps(x, gamma, w_q, w_kv, w_out, null_kv):
    x = np.asarray(x, dtype=np.float32)
    gamma = np.asarray(gamma, dtype=np.float32)
    w_q = np.asarray(w_q, dtype=np.float32)
    w_kv = np.asarray(w_kv, dtype=np.float32)
    w_out = np.asarray(w_out, dtype=np.float32)
    null_kv = np.asarray(null_kv, dtype=np.float32)

    g = gamma[:, None]
    wq_full = g * w_q * SCALE
    wk_full = g * w_kv[:, :INNER]
    wv_full = g * w_kv[:, INNER:]
    ident = np.eye(128, dtype=np.float16)

    in_maps = []
    for core in range(8):
        b, gr = core // 2, core % 2
        hs = slice(gr * HC, (gr + 1) * HC)
        nullk = np.zeros((128, NHEAD), dtype=np.float16)
        nullv = np.zeros((1, NHEAD, 65), dtype=np.float16)
        for j in range(NHEAD):
            h = gr * NHEAD + j
            ro = (j % 2) * 64
            nullk[ro:ro + 64, j] = null_kv[0, h, 0, :]
            nullv[0, j, :64] = null_kv[1, h, 0, :]
            nullv[0, j, 64] = 1.0
        in_maps.append({
            "x": np.ascontiguousarray(x[b]),
            "wq": np.ascontiguousarray(wq_full[:, hs]).astype(np.float16),
            "wk": np.ascontiguousarray(wk_full[:, hs]).astype(np.float16),
            "wv": np.ascontiguousarray(wv_full[:, hs]).astype(np.float16),
            "wo": np.ascontiguousarray(w_out[hs, :]).astype(np.float16),
            "nullk": nullk,
            "nullv": nullv,
            "ident": ident,
        })
    return in_maps


def kernel(x, gamma, w_q, w_kv, w_out, null_kv, _want_results=False):
    from concourse.bass_utils import run_bass_kernel_spmd

    nc = _get_nc()
    in_maps = _prep_in_maps(x, gamma, w_q, w_kv, w_out, null_kv)
    res = run_bass_kernel_spmd(nc, in_maps, list(range(8)))
    outs = [res.results[c]["y"] for c in range(8)]
    y = np.empty((4, N_TOK, DIM), dtype=np.float32)
    for b in range(4):
        np.add(outs[2 * b], outs[2 * b + 1], out=y[b])
    if _want_results:
        return y, res
    return y


# revision 25
# speedup vs baseline: 1.0711x; 1.0212x over previous
"""Trainium2 Bass kernel for pre-LN multi-head attention with null-KV.

Computation (fp32 reference):
    xn = LayerNorm(x) * gamma
    q = (xn @ w_q) * scale ; k, v = split(xn @ w_kv)
    k, v prepended with per-head null_kv
    out = softmax(q k^T) v   (16 heads, dim 64)
    y = out @ w_out

Sharding: 8 cores = 4 batches x 2 head-groups (8 heads each). Each core
computes LN for its batch, projections for its head group (gamma and the
q-scale folded into the weights on the host), attention, and a partial
out-projection; the host sums the two partial outputs per batch.

On-device layout is fully "transposed": xn is normalized to fp16 on the
gpsimd engine, transposed on the PE (fp16 transposes run 4x faster than
fp32), projections produce q^T / k^T directly, sim is computed as k q^T
(keys on partitions) so exp needs no cross-partition reductions, and
attn@v is computed with v (keys on partitions) as the stationary operand
with a fused ones-column yielding the softmax denominator in the same
matmul. The null key/value contribute one K=1 accumulation per head.
All matmul operands are fp16 (fp8 fails the accuracy budget: attention
output is itself an average, so quantization noise does not wash out
relative to the signal).
"""

import sys

sys.path.insert(0, "/opt/trn_rl_repo")

import numpy as np

HEADS = 16
DIM_HEAD = 64
DIM = 1024
INNER = HEADS * DIM_HEAD
SCALE = DIM_HEAD ** -0.5

N_TOK = 2048      # sequence length per batch
HC = 512          # head-cols per core (8 heads x 64)
NHEAD = 8         # heads per core
NPAIR = 4         # head pairs per core
NKT = 16          # key tiles of 128
NQC = 4           # query chunks of 512
NQUARTER = 4      # token quarters of 512 for the pre-phase
KC = 8            # contraction chunks of 128 over DIM

_CACHE: dict = {}


def _build_nc():
    from contextlib import ExitStack

    import concourse.bacc as bacc
    import concourse.bass as bass
    import concourse.tile as tile
    from concourse import mybir

    f32 = mybir.dt.float32
    f16 = mybir.dt.float16
    AF = mybir.ActivationFunctionType
    ALU = mybir.AluOpType
    PSUM = bass.MemorySpace.PSUM

    nc = bacc.Bacc(None)

    x_d = nc.declare_dram_parameter("x", [N_TOK, DIM], f32, isOutput=False)
    wq_d = nc.declare_dram_parameter("wq", [DIM, HC], f16, isOutput=False)
    wk_d = nc.declare_dram_parameter("wk", [DIM, HC], f16, isOutput=False)
    wv_d = nc.declare_dram_parameter("wv", [DIM, HC], f16, isOutput=False)
    wo_d = nc.declare_dram_parameter("wo", [HC, DIM], f16, isOutput=False)
    nullk_d = nc.declare_dram_parameter("nullk", [128, NHEAD], f16, isOutput=False)
    nullv_d = nc.declare_dram_parameter("nullv", [1, NHEAD, 65], f16, isOutput=False)
    ident_d = nc.declare_dram_parameter("ident", [128, 128], f16, isOutput=False)
    y_d = nc.declare_dram_parameter("y", [N_TOK, DIM], f32, isOutput=True)

    with tile.TileContext(nc) as tc, ExitStack() as ctx:
        pers = ctx.enter_context(tc.tile_pool(name="pers", bufs=1))

        ident_sb = pers.tile([128, 128], f16, tag="ident", name="ident")
        nc.sync.dma_start(out=ident_sb, in_=ident_d[:, :])
        eps_sb = pers.tile([128, 1], f32, tag="eps", name="eps")
        nc.vector.memset(eps_sb, 1e-5)

        nullk_sb = pers.tile([128, NHEAD], f16, tag="nullk", name="nullk")
        nc.sync.dma_start(out=nullk_sb, in_=nullk_d[:, :])
        nullv_sb = pers.tile([1, NHEAD, 65], f16, tag="nullv", name="nullv")
        nc.sync.dma_start(out=nullv_sb, in_=nullv_d[:, :, :])
        pnall = [pers.tile([1, 2, NQC, 512], f16, tag=f"pna{i}", name=f"pna{i}")
                 for i in range(NPAIR)]
        qT = [pers.tile([128, N_TOK], f16, tag=f"qT{i}", name=f"qT{i}") for i in range(NPAIR)]
        kT = [pers.tile([128, N_TOK], f16, tag=f"kT{i}", name=f"kT{i}") for i in range(NPAIR)]
        # flat [128, 584]: head h's v in cols h*65..h*65+64, the fused ones
        # column at h*65+64; 584 wide so a 128-col lhsT slice (FWL-eligible)
        # stays in bounds for every head
        v_sb = [pers.tile([128, 584], f16, tag=f"v{i}", name=f"v{i}") for i in range(NKT)]
        for t in range(NKT):
            nc.vector.memset(
                v_sb[t][:, 0:520].rearrange("p (h e) -> p h e", e=65)[:, :, 64:65],
                1.0)

        # ---------------- pre-phase: LN + transpose + projections ----------
        with (
            tc.tile_pool(name="wpool", bufs=1) as wpool,
            tc.tile_pool(name="xpool", bufs=8) as xpool,
            tc.tile_pool(name="xnpool", bufs=5) as xnpool,
            tc.tile_pool(name="xnt", bufs=3) as xnt_pool,
            tc.tile_pool(name="stat", bufs=4) as stat,
            tc.tile_pool(name="tps", bufs=4, space=PSUM) as tps,
            tc.tile_pool(name="pps", bufs=3, space=PSUM) as pps,
        ):
            # queue the first quarter's x tiles before the weight DMAs so
            # the LN/transpose pipeline starts immediately
            def dma_x(x_t, tt):
                nc.sync.dma_start(out=x_t,
                                  in_=x_d[tt * 128:(tt + 1) * 128, :])

            xq0 = []
            for t4 in range(4):
                x_t = xpool.tile([128, DIM], f32, tag="x", name="x0")
                dma_x(x_t, t4)
                xq0.append(x_t)
            wq_sb = [wpool.tile([128, HC], f16, tag=f"wq{k}", name=f"wq{k}") for k in range(KC)]
            wk_sb = [wpool.tile([128, HC], f16, tag=f"wk{k}", name=f"wk{k}") for k in range(KC)]
            wv_sb = [wpool.tile([128, HC], f16, tag=f"wv{k}", name=f"wv{k}") for k in range(KC)]
            for k in range(KC):
                sl = slice(k * 128, (k + 1) * 128)
                nc.sync.dma_start(out=wq_sb[k], in_=wq_d[sl, :])
                nc.sync.dma_start(out=wk_sb[k], in_=wk_d[sl, :])
            for k in range(KC):
                sl = slice(k * 128, (k + 1) * 128)
                nc.sync.dma_start(out=wv_sb[k], in_=wv_d[sl, :])

            def emit_nulls(Qn):
                # null-key logits for quarter Qn, emitted one quarter late so
                # the qT drain copies they read are long finished
                for ct in range(NPAIR):
                    for h2 in range(2):
                        h = ct * 2 + h2
                        ro = h2 * 64
                        pnp = pps.tile([128, 512], f32, tag="proj",
                                       name="pnp")
                        nc.tensor.matmul(
                            pnp[0:1, :],
                            lhsT=nullk_sb[ro:ro + 64, h:h + 1],
                            rhs=qT[ct][ro:ro + 64,
                                       Qn * 512:(Qn + 1) * 512],
                            start=True, stop=True)
                        nc.scalar.activation(
                            out=pnall[ct][0:1, h2, Qn, :],
                            in_=pnp[0:1, :], func=AF.Exp)

            for Q in range(NQUARTER):
                qcols = slice(Q * 512, (Q + 1) * 512)
                if Q > 0:
                    emit_nulls(Q - 1)
                xns = []
                for t4 in range(4):
                    tt = Q * 4 + t4
                    if Q == 0:
                        x_t = xq0[t4]
                    else:
                        x_t = xpool.tile([128, DIM], f32, tag="x", name="x")
                        dma_x(x_t, tt)
                    st6 = stat.tile([128, 2, 6], f32, tag="st6", name="st6")
                    nc.vector.bn_stats(out=st6[:, 0, :], in_=x_t[:, 0:512])
                    nc.vector.bn_stats(out=st6[:, 1, :], in_=x_t[:, 512:1024])
                    mv = stat.tile([128, 2], f32, tag="mv", name="mv")
                    nc.vector.bn_aggr(out=mv, in_=st6)
                    rstd = stat.tile([128, 1], f32, tag="rstd", name="rstd")
                    nc.scalar.activation(out=rstd, in_=mv[:, 1:2], func=AF.Sqrt,
                                         bias=eps_sb, scale=1.0)
                    nc.vector.reciprocal(out=rstd, in_=rstd)
                    xn_t = xnpool.tile([128, DIM], f16, tag="xn", name="xn")
                    nc.vector.tensor_scalar(out=xn_t, in0=x_t,
                                            scalar1=mv[:, 0:1], scalar2=rstd,
                                            op0=ALU.subtract, op1=ALU.mult)
                    xns.append(xn_t)

                # fp16 transposes, batched 4 token-tiles per psum tile so the
                # drain copy is [128, 512]
                xnT = [xnt_pool.tile([128, 512], f16, tag=f"xnT{k}",
                                     name=f"xnT{k}") for k in range(KC)]
                for fc in range(KC):
                    tp = tps.tile([128, 512], f16, tag="tp", name="tp")
                    for t4 in range(4):
                        nc.tensor.transpose(
                            tp[:, t4 * 128:(t4 + 1) * 128],
                            xns[t4][:, fc * 128:(fc + 1) * 128], ident_sb)
                    if fc % 2 == 0:
                        nc.scalar.copy(out=xnT[fc], in_=tp)
                    else:
                        nc.vector.tensor_copy(out=xnT[fc], in_=tp)

                # q^T and k^T projections for this token quarter
                for w_sb, dst, eng in ((wq_sb, qT, 0), (wk_sb, kT, 1)):
                    for ct in range(NPAIR):
                        ps = pps.tile([128, 512], f32, tag="proj", name="proj")
                        for k in range(KC):
                            nc.tensor.matmul(
                                ps,
                                lhsT=w_sb[k][:, ct * 128:(ct + 1) * 128],
                                rhs=xnT[k],
                                start=(k == 0), stop=(k == KC - 1))
                        if (ct + eng) % 2 == 0:
                            nc.vector.tensor_copy(out=dst[ct][:, qcols], in_=ps)
                        else:
                            nc.scalar.copy(out=dst[ct][:, qcols], in_=ps)
                # v projection (natural layout)
                for t4 in range(4):
                    tt = Q * 4 + t4
                    ps = pps.tile([128, 512], f32, tag="proj", name="projv")
                    for k in range(KC):
                        nc.tensor.matmul(
                            ps,
                            lhsT=xnT[k][:, t4 * 128:(t4 + 1) * 128],
                            rhs=wv_sb[k],
                            start=(k == 0), stop=(k == KC - 1))
                    nc.vector.tensor_copy(
                        out=v_sb[tt][:, 0:520].rearrange(
                            "p (h e) -> p h e", e=65)[:, :, 0:64],
                        in_=ps.rearrange("p (h d) -> p h d", h=NHEAD))

            emit_nulls(NQUARTER - 1)

        # ---------------- attention + out-projection ------------------------
        # Unit = 2 consecutive key tiles x both heads of a pair. Sims land in
        # 2-bank PSUM tiles (two matmuls share one tile-level semaphore) and
        # exp covers [128, 1024] per head, so PE-side waits are halved; the
        # attn@v stage trails by LAG units so the PE never blocks on exp.
        LAG = 1
        NU = NKT // 2
        with (
            tc.tile_pool(name="apool", bufs=1) as apool,
            tc.tile_pool(name="ppool", bufs=7) as ppool,
            tc.tile_pool(name="rcpool", bufs=2) as rcpool,
            tc.tile_pool(name="rbpool", bufs=3) as rbpool,
            tc.tile_pool(name="aspool", bufs=3) as aspool,
            tc.tile_pool(name="ypool", bufs=3) as ypool,
            tc.tile_pool(name="simy", bufs=3, space=PSUM) as simy,
            tc.tile_pool(name="accp", bufs=2, space=PSUM) as accps,
        ):
            outT = [apool.tile([128, N_TOK], f16, tag=f"outT{i}", name=f"outT{i}")
                    for i in range(NPAIR)]
            wo_sb = [apool.tile([128, DIM], f16, tag=f"wo{i}", name=f"wo{i}")
                     for i in range(NPAIR)]
            for i in range(NPAIR):
                nc.sync.dma_start(out=wo_sb[i], in_=wo_d[i * 128:(i + 1) * 128, :])

            def emit_yproj(c, t4):
                tt = c * 4 + t4
                for nh in range(2):
                    yp = simy.tile([128, 1024], f32, tag="simy", name="yp")
                    for ic in range(NPAIR):
                        nc.tensor.matmul(
                            yp[:, 0:512],
                            lhsT=outT[ic][:, tt * 128:(tt + 1) * 128],
                            rhs=wo_sb[ic][:, nh * 512:(nh + 1) * 512],
                            start=(ic == 0), stop=(ic == NPAIR - 1))
                    y_sb = ypool.tile([128, 512], f32, tag="ysb", name="ysb")
                    nc.vector.tensor_copy(out=y_sb, in_=yp[:, 0:512])
                    nc.sync.dma_start(
                        out=y_d[tt * 128:(tt + 1) * 128,
                                nh * 512:(nh + 1) * 512],
                        in_=y_sb)

            # accs are zeroed on the vector engine and every matmul
            # accumulates with start=False: the accumulation commutes, so
            # emission order around it is free — the null-kv init is emitted
            # late (after unit 0's sims) with the yproj matmuls as PE filler
            # during the first exp's latency, and each iteration's memsets
            # are hoisted into the previous iteration's tail so the DVE has
            # them done before the PE needs the acc.
            def fresh_accs():
                accs = []
                for h2 in range(2):
                    acc = accps.tile([128, 512], f32, tag="acc", name="acc")
                    nc.vector.memset(acc, 0.0)
                    accs.append(acc)
                return accs

            next_accs = fresh_accs()
            for c in range(NQC):
                ccols = slice(c * 512, (c + 1) * 512)
                for pr in range(NPAIR):
                    accs = next_accs
                    pend = []
                    for u in range(NU + LAG):
                        if u < NU:
                            sims = [simy.tile([128, 1024], f32, tag="simy",
                                              name="sim") for _ in range(2)]
                            for j in range(2):
                                kt = u * 2 + j
                                for h2 in range(2):
                                    ro = h2 * 64
                                    nc.tensor.matmul(
                                        sims[h2][:, j * 512:(j + 1) * 512],
                                        lhsT=kT[pr][ro:ro + 64,
                                                    kt * 128:(kt + 1) * 128],
                                        rhs=qT[pr][ro:ro + 64, ccols],
                                        start=True, stop=True)
                            ps2 = []
                            for h2 in range(2):
                                p_sb = ppool.tile([128, 1024], f16, tag="P",
                                                  name="P")
                                nc.scalar.activation(out=p_sb, in_=sims[h2],
                                                     func=AF.Exp)
                                ps2.append(p_sb)
                            pend.append(ps2)
                        if u == 0:
                            if c > 0:
                                emit_yproj(c - 1, pr)
                            for h2 in range(2):
                                h = pr * 2 + h2
                                nc.tensor.matmul(
                                    accs[h2][0:65, :],
                                    lhsT=nullv_sb[0:1, h, :],
                                    rhs=pnall[pr][0:1, h2, c, :],
                                    start=False, stop=False)
                        if u >= LAG:
                            ud = u - LAG
                            ps2 = pend[ud]
                            for h2 in range(2):
                                h = pr * 2 + h2
                                for j in range(2):
                                    kt = ud * 2 + j
                                    nc.tensor.matmul(
                                        accs[h2],
                                        lhsT=v_sb[kt][:, h * 65:h * 65 + 128],
                                        rhs=ps2[h2][:, j * 512:(j + 1) * 512],
                                        start=False, stop=(kt == NKT - 1))
                    # softmax normalization, off the critical path
                    if not (c == NQC - 1 and pr == NPAIR - 1):
                        next_accs = fresh_accs()
                    stg = []
                    for h2 in range(2):
                        a_s = aspool.tile([65, 512], f32, tag="accS",
                                          name="accS")
                        nc.vector.tensor_copy(out=a_s, in_=accs[h2][0:65, :])
                        z0 = rcpool.tile([1, 512], f32, tag="z0", name="z0")
                        nc.vector.tensor_copy(out=z0, in_=a_s[64:65, :])
                        rc = rcpool.tile([1, 512], f32, tag="rc", name="rc")
                        nc.vector.reciprocal_approx_fast(out=rc, in_=z0)
                        stg.append((a_s, rc))
                    rbs = []
                    for h2 in range(2):
                        rb = rbpool.tile([64, 512], f32, tag="rb", name="rb")
                        nc.gpsimd.partition_broadcast(rb, stg[h2][1],
                                                      channels=64)
                        rbs.append(rb)
                    for h2 in range(2):
                        ro = h2 * 64
                        nc.vector.tensor_mul(
                            out=outT[pr][ro:ro + 64, ccols],
                            in0=stg[h2][0][0:64, :], in1=rbs[h2])
            for t4 in range(4):
                emit_yproj(NQC - 1, t4)

    nc.compile()
    return nc


def _get_nc():
    if "nc" not in _CACHE:
        _CACHE["nc"] = _build_nc()
    return _CACHE["nc"]


def _prep_in_maps(x, gamma, w_q, w_kv, w_out, null_kv):
    x = np.asarray(x, dtype=np.float32)
    gamma = np.asarray(gamma, dtype=np.float32)
    w_q = np.asarray(w_q, dtype=np.float32)
    w_kv = np.asarray(w_kv, dtype=np.float32)
    w_out = np.asarray(w_out, dtype=np.float32)
    null_kv = np.asarray(null_kv, dtype=np.float32)

    g = gamma[:, None]
    wq_full = g * w_q * SCALE
    wk_full = g * w_kv[:, :INNER]
    wv_full = g * w_kv[:, INNER:]
    ident = np.eye(128, dtype=np.float16)

    in_maps = []
    for core in range(8):
        b, gr = core // 2, core % 2
        hs = slice(gr * HC, (gr + 1) * HC)
        nullk = np.zeros((128, NHEAD), dtype=np.float16)
        nullv = np.zeros((1, NHEAD, 65), dtype=np.float16)
        for j in range(NHEAD):
            h = gr * NHEAD + j
            ro = (j % 2) * 64
            nullk[ro:ro + 64, j] = null_kv[0, h, 0, :]
            nullv[0, j, :64] = null_kv[1, h, 0, :]
            nullv[0, j, 64] = 1.0
        in_maps.append({
            "x": np.ascontiguousarray(x[b]),
            "wq": np.ascontiguousarray(wq_full[:, hs]).astype(np.float16),
            "wk": np.ascontiguousarray(wk_full[:, hs]).astype(np.float16),
            "wv": np.ascontiguousarray(wv_full[:, hs]).astype(np.float16),
            "wo": np.ascontiguousarray(w_out[hs, :]).astype(np.float16),
            "nullk": nullk,
            "nullv": nullv,
            "ident": ident,
        })
    return in_maps


def kernel(x, gamma, w_q, w_kv, w_out, null_kv, _want_results=False):
    from concourse.bass_utils import run_bass_kernel_spmd

    nc = _get_nc()
    in_maps = _prep_in_maps(x, gamma, w_q, w_kv, w_out, null_kv)
    res = run_bass_kernel_spmd(nc, in_maps, list(range(8)))
    outs = [res.results[c]["y"] for c in range(8)]
    y = np.empty((4, N_TOK, DIM), dtype=np.float32)
    for b in range(4):
        np.add(outs[2 * b], outs[2 * b + 1], out=y[b])
    if _want_results:
        return y, res
    return y
